# revision 15
# baseline (speedup 1.0000x reference)
"""Trainium2 Bass kernel v3 for nn_Attention_40759239639448.

Contract: kernel(**inputs) takes full inputs, returns full [B, T, C] output.

v3 sharding: 2-way data-parallel on batch x 4-way tensor-parallel on heads
(cores 0-3 batch 0, cores 4-7 batch 1; 4 heads per core as two head-pairs).
This environment is shared-DMA-bandwidth bound (~100 GB/s aggregate across
the 8 cores), so v3 minimizes per-core HBM traffic (~11MB vs v2's ~21MB):
 - x: one batch only, fp16 [C, T] (4MB vs 8MB)
 - out partial: one batch only, fp16 (4MB vs 8MB); the two head-pairs'
   out-projections chain into one PSUM accumulation so there is a single
   store per q-tile
 - trig tables: fp16 [64, 4*T] (1MB vs 4MB f32), broadcast 64->128
   partitions on-chip via one PE matmul per segment
 - weights fp16 (2MB, 4 heads' worth)

Compute structure per core: the two head-pairs play the role v2's two
batches played; chunk ci (512 tokens) runs QKV+RoPE+RMS for both pairs,
then causal attention q-tile ci for both pairs, with v2's software
pipelining (next-chunk projection groups threaded as PE filler into the
attention k-loops).
"""
import sys

sys.path.insert(0, "/opt/trn_rl_repo")

import numpy as np
import ml_dtypes

F16 = np.float16

B, T, C, H = 2, 2048, 1024, 16
D = C // H
NC = 8
EPS = 1e-6
ROPE_BASE = 10000.0
NCH = 4          # token chunks of 512 per batch
CH = 512
KT = 128
QT = 512
NKT = T // KT    # 16
SC = 0.125
NEG = -30000.0   # causal bias; fp16-safe, exp(SC*(-30000+s)) == 0

_cache = {}


def _patch_act_tables():
    """Route {exp, ln, square, copy, identity, memset_zero} exclusively to
    natural_log_exp_and_others so the table-load pass can't thrash between
    sets. Names/indices preserved; only membership shrinks."""
    import concourse.hw_specs as hw_specs
    import concourse.bacc as bacc
    import concourse.mybir as mybir

    AF = mybir.ActivationFunctionType
    mine = {AF.Exp, AF.Ln, AF.Square, AF.Copy, AF.Identity, AF.MemsetZero}
    orig = hw_specs.get_activation_tables
    if getattr(hw_specs, "_act_tables_patched", False):
        return

    def patched(module_arch):
        tables = orig(module_arch)
        if not any(
            name == "natural_log_exp_and_others" and mine <= fns
            for name, fns in tables.items()
        ):
            return tables
        return {
            name: (fns if name == "natural_log_exp_and_others" else fns - mine)
            for name, fns in tables.items()
        }

    hw_specs.get_activation_tables = patched
    bacc.get_activation_tables = patched
    hw_specs._act_tables_patched = True


def _build(reps=1):
    _patch_act_tables()
    from contextlib import nullcontext

    import concourse.bacc as bacc
    import concourse.mybir as mybir
    import concourse.tile as tile

    f32 = mybir.dt.float32
    f32r = mybir.dt.float32r
    f16 = mybir.dt.float16
    AF = mybir.ActivationFunctionType

    nc = bacc.Bacc(None, target_bir_lowering=False)

    xT_d = nc.dram_tensor("xT", [C, T], f16, kind="ExternalInput")
    # weights pre-permuted on host into SBUF layout [128, pair, group, 128]
    wqT_d = nc.dram_tensor("wqT", [128, 2 * C], f16, kind="ExternalInput")
    wkT_d = nc.dram_tensor("wkT", [128, 2 * C], f16, kind="ExternalInput")
    wvT_d = nc.dram_tensor("wvT", [128, 2 * C], f16, kind="ExternalInput")
    woT_d = nc.dram_tensor("woT", [128, 2 * C], f16, kind="ExternalInput")
    trig_d = nc.dram_tensor("trig", [64, 4 * T], f16, kind="ExternalInput")
    bc_d = nc.dram_tensor("bc", [64, 128], f16, kind="ExternalInput")
    mtri_d = nc.dram_tensor("mtri", [128, 128], f16, kind="ExternalInput")
    ee_d = nc.dram_tensor("ee", [128, 128], f32r, kind="ExternalInput")
    ident_d = nc.dram_tensor("ident", [128, 128], f16, kind="ExternalInput")
    onesc_d = nc.dram_tensor("onesc", [1, 64], f32r, kind="ExternalInput")
    epsb_d = nc.dram_tensor("epsb", [128, 1], f32, kind="ExternalInput")
    out_d = nc.dram_tensor("out", [T, C], f16, kind="ExternalOutput")

    # rope-pair partner is 16 partitions away inside each 32-quadrant
    # (host permutes the qk d-channel order to arrange this)
    SHUF_MASK = [i ^ 16 for i in range(32)]

    with tile.TileContext(nc) as tc:
        with (
            tc.tile_pool(name="persist", bufs=1) as pp,
            tc.tile_pool(name="xp", bufs=3) as xp,
            tc.tile_pool(name="scr", bufs=8) as scr,
            tc.tile_pool(name="scrv", bufs=2) as scrv,
            tc.tile_pool(name="scr2", bufs=2) as scr2,
            tc.tile_pool(name="pbuf", bufs=10) as pbuf,
            tc.tile_pool(name="ysc", bufs=6) as ysc,
            tc.tile_pool(name="osb", bufs=4) as osb,
            tc.tile_pool(name="ps_c", bufs=2, space="PSUM") as ps_c,
            tc.tile_pool(name="ps_s", bufs=2, space="PSUM") as ps_s,
            tc.tile_pool(name="ps_y", bufs=1, space="PSUM") as ps_y,
        ):
            # persistent state; pair-major column blocks
            qT = pp.tile([128, 2 * T], f16, tag="qT")
            kT = pp.tile([128, 2 * T], f16, tag="kT")
            vaug = pp.tile([128, 2 * NKT * 130], f16, tag="vaug")
            wq_sb = pp.tile([128, 2 * C], f16, tag="wq")
            wk_sb = pp.tile([128, 2 * C], f16, tag="wk")
            wv_sb = pp.tile([128, 2 * C], f16, tag="wv")
            wo_sb = pp.tile([128, 2 * C], f16, tag="wo")
            trig = pp.tile([128, 4 * T], f16, tag="trig")
            bc = pp.tile([64, 128], f16, tag="bc")
            mtri = pp.tile([128, 128], f16, tag="mtri")
            ee = pp.tile([128, 128], f32r, tag="ee")
            ident = pp.tile([128, 128], f16, tag="ident")
            onesc = pp.tile([1, 64], f32r, tag="onesc")
            epsb = pp.tile([128, 1], f32, tag="epsb")

            # weight/const DMAs off the critical SP queue (x loads own it);
            # host pre-permuted to SBUF layout: straight 4KB/partition copies
            nc.scalar.dma_start(wq_sb[:], wqT_d[:])
            nc.scalar.dma_start(wk_sb[:], wkT_d[:])
            nc.scalar.dma_start(wv_sb[:], wvT_d[:])
            nc.scalar.dma_start(wo_sb[:], woT_d[:])
            nc.gpsimd.dma_start(trig[0:64, :], trig_d[:])
            nc.gpsimd.dma_start(bc[:], bc_d[:])
            nc.gpsimd.dma_start(mtri[:], mtri_d[:])
            nc.gpsimd.dma_start(ee[:], ee_d[:])
            nc.gpsimd.dma_start(ident[:], ident_d[:])
            nc.gpsimd.dma_start(onesc[:], onesc_d[:])
            nc.gpsimd.dma_start(epsb[:], epsb_d[:])
            nc.gpsimd.memset(
                vaug[:].rearrange("p (k c) -> p k c", c=65)[:, :, 64], 1.0
            )

            loop_cm = tc.For_i(0, reps) if reps > 1 else nullcontext()
            with loop_cm:
                # broadcast trig [64, 4T] -> [128, 4T] in place via PE
                # (bc = [I64|I64]; rows 0-63 rewritten with equal values)
                def trig_bcast(seg):
                    def go():
                        bp = ps_s.tile([128, 1024], f32, tag="s")
                        for j in range(2):
                            nc.tensor.matmul(
                                bp[:, j * 512 : j * 512 + 512],
                                bc[:],
                                trig[
                                    0:64,
                                    seg * 1024 + j * 512 : seg * 1024 + j * 512 + 512,
                                ],
                                start=True, stop=True,
                            )
                        nc.scalar.copy(
                            trig[:, seg * 1024 : seg * 1024 + 1024], bp[:]
                        )
                    return go

                def phase1_groups(ci, pair, load_x):
                    """Issue-groups for (chunk ci, head-pair pair)'s
                    QKV+RoPE+RMS. x chunk is shared: loaded once (pair 0)."""
                    t0 = ci * CH
                    state = {}

                    if load_x:
                        xt = xp.tile([128, 8, CH], f16, tag="x")
                        xsrc = xT_d[:, t0 : t0 + CH].rearrange(
                            "(g p) t -> p g t", p=128
                        )
                        nc.sync.dma_start(xt[:, 0:4, :], xsrc[:, 0:4, :])
                        nc.sync.dma_start(xt[:, 4:8, :], xsrc[:, 4:8, :])
                        state["xt"] = xt
                        _xt_share[ci] = xt
                    else:
                        state["xt"] = _xt_share[ci]

                    def tslc(w):
                        # trig table w: 0=cosq 1=sinq 2=cosk 3=sink
                        return trig[:, w * T + t0 : w * T + t0 + CH]

                    def proj_a(w_sb, name):
                        def go():
                            ps = ps_c.tile([128, CH], f32, tag="c")
                            state[name] = ps
                            for cc in range(4):
                                nc.tensor.matmul(
                                    ps[:],
                                    w_sb[:, pair * C + 128 * cc : pair * C + 128 * cc + 128],
                                    state["xt"][:, cc, :],
                                    start=(cc == 0), stop=False,
                                )
                        return go

                    def proj(w_sb, name):
                        def go():
                            ps = state[name]
                            for cc in range(4, 8):
                                nc.tensor.matmul(
                                    ps[:],
                                    w_sb[:, pair * C + 128 * cc : pair * C + 128 * cc + 128],
                                    state["xt"][:, cc, :],
                                    start=False, stop=(cc == 7),
                                )
                            if name != "v":
                                cw = 0 if name == "q" else 2
                                xs = scr.tile([128, CH], f32, tag="s")
                                nc.vector.stream_shuffle(xs[:], ps[:], SHUF_MASK)
                                tc_ = scr.tile([128, CH], f32, tag="s")
                                nc.vector.tensor_mul(tc_[:], ps[:], tslc(cw))
                                # sum of squares is pair-permutation
                                # invariant: square the shuffled SBUF copy
                                sq = scr.tile([128, CH], f32r, tag="s")
                                with nc.allow_low_precision(reason="rms f32r"):
                                    nc.vector.tensor_mul(sq[:], xs[:], xs[:])
                                ts2 = scr.tile([128, CH], f32, tag="s")
                                nc.gpsimd.tensor_mul(ts2[:], xs[:], tslc(cw + 1))
                                o_ = scr.tile([128, CH], f32, tag="s")
                                nc.gpsimd.tensor_add(o_[:], tc_[:], ts2[:])
                                state[name + "_sq"] = sq
                                state[name + "_o"] = o_
                            else:
                                v_sb = scrv.tile([128, CH], f16, tag="v")
                                if (2 * ci + pair + 3) % 4 < 2:
                                    nc.scalar.copy(v_sb[:], ps[:])
                                else:
                                    nc.vector.tensor_copy(v_sb[:], ps[:])
                                state["v_sb"] = v_sb

                        return go

                    def rms_and_vt():
                        # block-identity matmul broadcasts sum(q^2) per head;
                        # q and k share one tile: ln/exp run once, full width
                        ms_ps = ps_s.tile([128, 2 * CH], f32, tag="s")
                        for i, name in enumerate(("q", "k")):
                            nc.tensor.matmul(
                                ms_ps[:, i * CH : i * CH + CH],
                                ee[:], state[name + "_sq"][:],
                                start=True, stop=True,
                            )
                        state["ms"] = ms_ps
                        for ti in range(4):
                            vt_ps = ps_c.tile([128, 128], f16, tag="c")
                            nc.tensor.transpose(
                                vt_ps[:],
                                state["v_sb"][:, ti * 128 : ti * 128 + 128],
                                ident[:],
                            )
                            kb = pair * NKT + (t0 // KT) + ti
                            dst = vaug[:, kb * 130 : kb * 130 + 130].rearrange(
                                "p (k c) -> p k c", c=65
                            )[:, :, 0:64]
                            nc.vector.tensor_copy(
                                dst, vt_ps[:].rearrange("p (k c) -> p k c", c=64)
                            )

                    def norm_store():
                        ln_ = scr2.tile([128, 2 * CH], f32, tag="s2")
                        nc.scalar.activation(
                            ln_[:], state["ms"][:], AF.Ln,
                            scale=1.0 / D, bias=epsb[:],
                        )
                        rs_ = scr2.tile([128, 2 * CH], f32, tag="s2")
                        nc.scalar.activation(rs_[:], ln_[:], AF.Exp, scale=-0.5)
                        for i, (name, dst) in enumerate((("q", qT), ("k", kT))):
                            nc.vector.tensor_mul(
                                dst[:, pair * T + t0 : pair * T + t0 + CH],
                                state[name + "_o"][:],
                                rs_[:, i * CH : i * CH + CH],
                            )

                    return [
                        proj_a(wq_sb, "q"),
                        proj(wq_sb, "q"),
                        proj_a(wk_sb, "k"),
                        proj(wk_sb, "k"),
                        proj_a(wv_sb, "v"),
                        proj(wv_sb, "v"),
                        rms_and_vt,
                        norm_store,
                    ]

                def attn(pair, qi, fillers):
                    """Attention k-loop for (pair, qi), popping one filler
                    closure after each (s_exp, pv) pair so PE always has
                    independent work queued behind the exp-gated PV
                    matmuls."""
                    bt = pair * T
                    q0 = bt + qi * QT
                    nk = 4 * qi + 4
                    y_ps = ps_y.tile([65, 2 * QT], f32, tag="y")
                    p_tiles = [None] * nk
                    fillers = list(fillers)

                    def fill():
                        if fillers:
                            fillers.pop(0)()

                    def s_exp(ki):
                        mi = ki - 4 * qi
                        off = max(mi, 0) * KT
                        diag = mi >= 0
                        k0 = bt + ki * KT
                        s_ps = ps_s.tile([128, 2 * QT], f32, tag="s")
                        for h in range(2):
                            hs = slice(64 * h, 64 * h + 64)
                            nc.tensor.matmul(
                                s_ps[:, h * QT + off : h * QT + QT],
                                kT[hs, k0 : k0 + KT],
                                qT[hs, q0 + off : q0 + QT],
                                start=True, stop=not diag,
                                tile_position=(64 * h, 0),
                            )
                        if diag:
                            # causal mask: accumulate NEG strict-lower
                            # triangle into the diagonal block; exp -> 0
                            for h in range(2):
                                nc.tensor.matmul(
                                    s_ps[:, h * QT + off : h * QT + off + KT],
                                    mtri[:], ident[:],
                                    start=False, stop=True,
                                )
                        p_sb = pbuf.tile([128, 2 * QT], f16, tag="p")
                        if off == 0:
                            nc.scalar.activation(
                                p_sb[:], s_ps[:], AF.Exp, scale=SC
                            )
                        else:
                            nc.scalar.activation(
                                p_sb[:].rearrange("p (h q) -> p h q", h=2)[
                                    :, :, off:QT
                                ],
                                s_ps[:].rearrange("p (h q) -> p h q", h=2)[
                                    :, :, off:QT
                                ],
                                AF.Exp, scale=SC,
                            )
                        p_tiles[ki] = p_sb

                    def pv(ki):
                        mi = ki - 4 * qi
                        off = max(mi, 0) * KT
                        kb = pair * NKT + ki
                        for h in range(2):
                            nc.tensor.matmul(
                                y_ps[:, h * QT + off : h * QT + QT],
                                vaug[
                                    :, kb * 130 + 65 * h : kb * 130 + 65 * h + 65
                                ],
                                p_tiles[ki][:, h * QT + off : h * QT + QT],
                                start=(ki == 0), stop=(ki == nk - 1),
                            )

                    # spread fillers evenly over the k-loop's fill slots
                    nslots = nk + 1
                    nf = len(fillers)
                    sched = {}
                    for i in range(nf):
                        s = min(nslots - 1, (i * nslots) // max(nf, 1))
                        sched[s] = sched.get(s, 0) + 1
                    slot = 0

                    def fill_at():
                        nonlocal slot
                        for _ in range(sched.get(slot, 0)):
                            fill()
                        slot += 1

                    s_exp(0)
                    if nk > 1:
                        s_exp(1)
                    fill_at()
                    for ki in range(2, nk):
                        s_exp(ki)
                        fill_at()
                        pv(ki - 2)
                    fill_at()
                    pv(nk - 2) if nk > 1 else None
                    pv(nk - 1)
                    while fillers:
                        fill()

                    # free y_ps immediately; normalization defers as fillers
                    yraw = ysc.tile([65, 2 * QT], f32r, tag="yraw")
                    nc.scalar.copy(yraw[64:65, :], y_ps[64:65, :])
                    if qi == 3 and pair == 1:
                        nc.scalar.copy(yraw[0:64, :], y_ps[0:64, :])
                    else:
                        nc.vector.tensor_copy(yraw[0:64, :], y_ps[0:64, :])
                    rcp = ysc.tile([1, 2 * QT], f32r, tag="rcp")
                    with nc.allow_low_precision(reason="denominator recip"):
                        nc.vector.reciprocal(rcp[:], yraw[64:65, :])

                    def tail_norm():
                        db_ps = ps_y.tile([64, 2 * QT], f32, tag="y")
                        for h in range(2):
                            nc.tensor.matmul(
                                db_ps[:, h * QT : h * QT + QT],
                                onesc[:],
                                rcp[:, h * QT : h * QT + QT],
                                start=True, stop=True,
                            )
                        yTq = ysc.tile([128, QT], f16, tag="yTq")
                        for h in range(2):
                            nc.vector.tensor_mul(
                                yTq[64 * h : 64 * h + 64, :],
                                yraw[0:64, h * QT : h * QT + QT],
                                db_ps[:, h * QT : h * QT + QT],
                            )
                        yTq_store[pair] = yTq

                    return tail_norm

                def tail_proj(qi, yTq0, yTq1, last):
                    """Out-projection for q-tile qi: both pairs chained into
                    one PSUM accumulation (contraction 256), single copy and
                    one DMA per 128-token block."""
                    def go():
                        q0 = qi * QT
                        for tt in range(4):
                            o_sb = osb.tile([128, C], f16, tag="o")
                            for half in range(2):
                                o_ps = ps_c.tile([128, 512], f32, tag="c")
                                nc.tensor.matmul(
                                    o_ps[:],
                                    yTq0[:, tt * 128 : tt * 128 + 128],
                                    wo_sb[:, half * 512 : half * 512 + 512],
                                    start=True, stop=False,
                                )
                                nc.tensor.matmul(
                                    o_ps[:],
                                    yTq1[:, tt * 128 : tt * 128 + 128],
                                    wo_sb[:, C + half * 512 : C + half * 512 + 512],
                                    start=False, stop=True,
                                )
                                dsl = o_sb[:, half * 512 : half * 512 + 512]
                                if half == 0 or last:
                                    nc.scalar.copy(dsl, o_ps[:])
                                else:
                                    nc.vector.tensor_copy(dsl, o_ps[:])
                            nc.gpsimd.dma_start(
                                out_d[q0 + tt * 128 : q0 + tt * 128 + 128, :],
                                o_sb[:],
                            )
                    return go

                _xt_share = {}
                yTq_store = {}

                # startup: chunk 0 for both pairs runs un-overlapped, with
                # the trig broadcast segments woven in front (tables must
                # exist before the rope muls)
                for g in [trig_bcast(s) for s in range(8)]:
                    g()
                for g in phase1_groups(0, 0, True):
                    g()
                for g in phase1_groups(0, 1, False):
                    g()

                # steady state: per chunk ci, attention for both pairs at
                # q-tile ci; phase1 for chunk ci+1 and the previous q-tile's
                # tails thread in as PE filler.
                tails = []      # deferred closures from previous iterations
                for ci in range(NCH):
                    nxt = (
                        phase1_groups(ci + 1, 0, True)
                        + phase1_groups(ci + 1, 1, False)
                        if ci + 1 < NCH
                        else []
                    )
                    # attn(pair0): fillers = tails + first half of phase1
                    f0 = tails + nxt[0:8]
                    tn0 = attn(0, ci, f0)
                    # attn(pair1): fillers = pair0's tail_norm + rest
                    f1 = [tn0] + nxt[8:16]
                    tn1 = attn(1, ci, f1)
                    # defer pair1 tail_norm + fused out-projection into the
                    # next iteration's pair-0 k-loop (it is long enough)
                    y0ref = []

                    def mk_tail(ci=ci, tn1=tn1):
                        done = {}

                        def run_tn1():
                            tn1()
                            done["y0"] = yTq_store[0]
                            done["y1"] = yTq_store[1]

                        def run_proj():
                            tail_proj(
                                ci, done["y0"], done["y1"], ci == NCH - 1
                            )()

                        return [run_tn1, run_proj]

                    tails = mk_tail()
                for g in tails:
                    g()

    nc.finalize()
    return nc


def _host_prep(x, w_qkv, w_out, q_norm_w, k_norm_w):
    j = np.arange(32, dtype=np.float64)
    inv = ROPE_BASE ** (-j / 32.0)
    tt = np.arange(T, dtype=np.float64)
    ang = tt[:, None] * inv[None, :]
    cos_t = np.cos(ang)
    sin_t = np.sin(ang)

    # d-channel order per head: rope pair (d, d+32) sits 16 partitions
    # apart within one 32-partition quadrant (stream_shuffle i^16 reaches it)
    dmap = np.r_[0:16, 32:48, 16:32, 48:64]

    def trig_rows(w):
        w = np.asarray(w, dtype=np.float64)
        cosr = np.empty((64, T), np.float64)
        sinr = np.empty((64, T), np.float64)
        for p in range(64):
            d = dmap[p]
            jj = d % 32
            sign = -1.0 if d < 32 else 1.0
            cosr[p] = cos_t[:, jj] * w[d]
            sinr[p] = sign * sin_t[:, jj] * w[d]
        return cosr, sinr

    cq, sq = trig_rows(q_norm_w)
    ck, sk = trig_rows(k_norm_w)
    trig = np.concatenate([cq, sq, ck, sk], axis=1).astype(F16)

    bc = np.zeros((64, 128), np.float32)
    bc[np.arange(64), np.arange(64)] = 1.0
    bc[np.arange(64), 64 + np.arange(64)] = 1.0

    mtri = np.where(
        np.arange(128)[:, None] < np.arange(128)[None, :], NEG, 0.0
    ).astype(F16)
    ee = np.zeros((128, 128), np.float32)
    ee[0:64, 0:64] = 1.0
    ee[64:128, 64:128] = 1.0
    ident = np.eye(128, dtype=np.float32).astype(F16)

    shared = {
        "trig": trig, "bc": bc.astype(F16), "mtri": mtri, "ee": ee,
        "ident": ident,
        "onesc": np.ones((1, 64), np.float32),
        "epsb": np.full((128, 1), EPS, np.float32),
    }

    xTb = [
        np.ascontiguousarray(x[b].T.astype(F16)) for b in range(B)
    ]

    in_maps = []
    for c in range(NC):
        bb = c // 4
        hg = c % 4
        m = dict(shared)
        m["xT"] = xTb[bb]
        # two head-pairs: rows for pair p = heads (4hg+2p, 4hg+2p+1)
        qk_rows = []
        v_rows = []
        for p in range(2):
            h0 = (4 * hg + 2 * p) * 64
            rows = np.arange(h0, h0 + 128)
            qk_rows.append(np.concatenate([rows[0:64][dmap], rows[64:128][dmap]]))
            v_rows.append(rows)
        qk_rows = np.concatenate(qk_rows)   # [256]
        v_rows = np.concatenate(v_rows)     # [256]
        # proj matmul lhsT layout: wq_sb[p_in, q*C + g*128 + i_out]
        #   = w_qkv[row_set[q*128 + i_out], g*128 + p_in]
        def sbuf_w(wmat, rows):
            arr = wmat[rows, :].reshape(2, 128, 8, 128)  # [q, i, g, p]
            return np.ascontiguousarray(
                arr.transpose(3, 0, 2, 1).reshape(128, 2 * C).astype(F16)
            )

        m["wqT"] = sbuf_w(w_qkv, qk_rows)
        m["wkT"] = sbuf_w(w_qkv, C + qk_rows)
        m["wvT"] = sbuf_w(w_qkv, 2 * C + v_rows)
        # wo_sb[p, q*C + c] = w_out[c, v_rows[q*128 + p]]
        m["woT"] = np.ascontiguousarray(
            w_out[:, v_rows].T.reshape(2, 128, C).transpose(1, 0, 2)
            .reshape(128, 2 * C).astype(F16)
        )
        in_maps.append(m)
    return in_maps


def kernel(x, w_qkv, w_out, q_norm_w, k_norm_w, _trace=False):
    from concourse.bass_utils import run_bass_kernel_spmd

    if "nc" not in _cache:
        _cache["nc"] = _build()
    nc = _cache["nc"]

    x = np.asarray(x, dtype=np.float32)
    w_qkv = np.asarray(w_qkv, dtype=np.float32)
    w_out = np.asarray(w_out, dtype=np.float32)
    q_norm_w = np.asarray(q_norm_w, dtype=np.float32)
    k_norm_w = np.asarray(k_norm_w, dtype=np.float32)

    in_maps = _host_prep(x, w_qkv, w_out, q_norm_w, k_norm_w)
    res = run_bass_kernel_spmd(nc, in_maps, list(range(NC)), trace=_trace)
    _cache["last_result"] = res
    out = np.zeros((B, T, C), np.float64)
    for c, r in enumerate(res.results):
        out[c // 4] += np.asarray(r["out"]).astype(np.float64)
    return out.astype(np.float32)


# revision 24
# speedup vs baseline: 1.0199x; 1.0199x over previous
"""Trainium2 Bass kernel v3 for nn_Attention_40759239639448.

Contract: kernel(**inputs) takes full inputs, returns full [B, T, C] output.

v3 sharding: 2-way data-parallel on batch x 4-way tensor-parallel on heads
(cores 0-3 batch 0, cores 4-7 batch 1; 4 heads per core as two head-pairs).
This environment is shared-DMA-bandwidth bound (~100 GB/s aggregate across
the 8 cores), so v3 minimizes per-core HBM traffic (~11MB vs v2's ~21MB):
 - x: one batch only, fp16 [C, T] (4MB vs 8MB)
 - out partial: one batch only, fp16 (4MB vs 8MB); the two head-pairs'
   out-projections chain into one PSUM accumulation so there is a single
   store per q-tile
 - trig tables: fp16 [64, 4*T] (1MB vs 4MB f32), broadcast 64->128
   partitions on-chip via one PE matmul per segment
 - weights fp16 (2MB, 4 heads' worth)

Compute structure per core: the two head-pairs play the role v2's two
batches played; chunk ci (512 tokens) runs QKV+RoPE+RMS for both pairs,
then causal attention q-tile ci for both pairs, with v2's software
pipelining (next-chunk projection groups threaded as PE filler into the
attention k-loops).
"""
import sys

sys.path.insert(0, "/opt/trn_rl_repo")

import numpy as np
import ml_dtypes

F16 = np.float16

B, T, C, H = 2, 2048, 1024, 16
D = C // H
NC = 8
EPS = 1e-6
ROPE_BASE = 10000.0
NCH = 4          # token chunks of 512 per batch
CH = 512
KT = 128
QT = 512
NKT = T // KT    # 16
SC = 0.125
NEG = -30000.0   # causal bias; fp16-safe, exp(SC*(-30000+s)) == 0

_cache = {}


def _patch_act_tables():
    """Route {exp, ln, square, copy, identity, memset_zero} exclusively to
    natural_log_exp_and_others so the table-load pass can't thrash between
    sets. Names/indices preserved; only membership shrinks."""
    import concourse.hw_specs as hw_specs
    import concourse.bacc as bacc
    import concourse.mybir as mybir

    AF = mybir.ActivationFunctionType
    mine = {AF.Exp, AF.Ln, AF.Square, AF.Copy, AF.Identity, AF.MemsetZero}
    orig = hw_specs.get_activation_tables
    if getattr(hw_specs, "_act_tables_patched", False):
        return

    def patched(module_arch):
        tables = orig(module_arch)
        if not any(
            name == "natural_log_exp_and_others" and mine <= fns
            for name, fns in tables.items()
        ):
            return tables
        return {
            name: (fns if name == "natural_log_exp_and_others" else fns - mine)
            for name, fns in tables.items()
        }

    hw_specs.get_activation_tables = patched
    bacc.get_activation_tables = patched
    hw_specs._act_tables_patched = True


def _build(reps=1):
    _patch_act_tables()
    from contextlib import nullcontext

    import concourse.bacc as bacc
    import concourse.mybir as mybir
    import concourse.tile as tile

    f32 = mybir.dt.float32
    f32r = mybir.dt.float32r
    f16 = mybir.dt.float16
    AF = mybir.ActivationFunctionType

    nc = bacc.Bacc(None, target_bir_lowering=False)

    xT_d = nc.dram_tensor("xT", [C, T], f16, kind="ExternalInput")
    # weights pre-permuted on host into SBUF layout [128, pair, group, 128]
    wqT_d = nc.dram_tensor("wqT", [128, 2 * C], f16, kind="ExternalInput")
    wkT_d = nc.dram_tensor("wkT", [128, 2 * C], f16, kind="ExternalInput")
    wvT_d = nc.dram_tensor("wvT", [128, 2 * C], f16, kind="ExternalInput")
    woT_d = nc.dram_tensor("woT", [128, 2 * C], f16, kind="ExternalInput")
    trig_d = nc.dram_tensor("trig", [64, 2 * T], f16, kind="ExternalInput")
    wnorm_d = nc.dram_tensor("wnorm", [128, 2], f32, kind="ExternalInput")
    bc_d = nc.dram_tensor("bc", [64, 128], f16, kind="ExternalInput")
    mtri_d = nc.dram_tensor("mtri", [128, 128], f16, kind="ExternalInput")
    ee_d = nc.dram_tensor("ee", [128, 128], f32r, kind="ExternalInput")
    ident_d = nc.dram_tensor("ident", [128, 128], f16, kind="ExternalInput")
    onesc_d = nc.dram_tensor("onesc", [1, 64], f32r, kind="ExternalInput")
    epsb_d = nc.dram_tensor("epsb", [128, 1], f32, kind="ExternalInput")
    out_d = nc.dram_tensor("out", [T, C], f16, kind="ExternalOutput")

    # rope-pair partner is 16 partitions away inside each 32-quadrant
    # (host permutes the qk d-channel order to arrange this)
    SHUF_MASK = [i ^ 16 for i in range(32)]

    with tile.TileContext(nc) as tc:
        with (
            tc.tile_pool(name="persist", bufs=1) as pp,
            tc.tile_pool(name="xp", bufs=3) as xp,
            tc.tile_pool(name="scr", bufs=8) as scr,
            tc.tile_pool(name="scrv", bufs=2) as scrv,
            tc.tile_pool(name="scr2", bufs=2) as scr2,
            tc.tile_pool(name="pbuf", bufs=10) as pbuf,
            tc.tile_pool(name="ysc", bufs=6) as ysc,
            tc.tile_pool(name="osb", bufs=4) as osb,
            tc.tile_pool(name="ps_c", bufs=2, space="PSUM") as ps_c,
            tc.tile_pool(name="ps_s", bufs=2, space="PSUM") as ps_s,
            tc.tile_pool(name="ps_y", bufs=1, space="PSUM") as ps_y,
        ):
            # persistent state; pair-major column blocks
            qT = pp.tile([128, 2 * T], f16, tag="qT")
            kT = pp.tile([128, 2 * T], f16, tag="kT")
            vaug = pp.tile([128, 2 * NKT * 130], f16, tag="vaug")
            wq_sb = pp.tile([128, 2 * C], f16, tag="wq")
            wk_sb = pp.tile([128, 2 * C], f16, tag="wk")
            wv_sb = pp.tile([128, 2 * C], f16, tag="wv")
            wo_sb = pp.tile([128, 2 * C], f16, tag="wo")
            trig = pp.tile([128, 2 * T], f16, tag="trig")
            wnorm = pp.tile([128, 2], f32, tag="wnorm")
            bc = pp.tile([64, 128], f16, tag="bc")
            mtri = pp.tile([128, 128], f16, tag="mtri")
            ee = pp.tile([128, 128], f32r, tag="ee")
            ident = pp.tile([128, 128], f16, tag="ident")
            onesc = pp.tile([1, 64], f32r, tag="onesc")
            epsb = pp.tile([128, 1], f32, tag="epsb")

            loop_cm = tc.For_i(0, reps) if reps > 1 else nullcontext()
            with loop_cm:
                # weight/const DMAs off the critical SP queue (x loads own
                # it); host pre-permuted to SBUF layout: 4KB/partition copies
                nc.scalar.dma_start(wq_sb[:], wqT_d[:])
                nc.scalar.dma_start(wk_sb[:], wkT_d[:])
                nc.scalar.dma_start(wv_sb[:], wvT_d[:])
                nc.scalar.dma_start(wo_sb[:], woT_d[:])
                nc.gpsimd.dma_start(trig[0:64, :], trig_d[:])
                nc.gpsimd.dma_start(wnorm[:], wnorm_d[:])
                nc.gpsimd.dma_start(bc[:], bc_d[:])
                nc.gpsimd.dma_start(mtri[:], mtri_d[:])
                nc.gpsimd.dma_start(ee[:], ee_d[:])
                nc.gpsimd.dma_start(ident[:], ident_d[:])
                nc.gpsimd.dma_start(onesc[:], onesc_d[:])
                nc.gpsimd.dma_start(epsb[:], epsb_d[:])
                nc.gpsimd.memset(
                    vaug[:].rearrange("p (k c) -> p k c", c=65)[:, :, 64], 1.0
                )
                # broadcast trig [64, 4T] -> [128, 4T] in place via PE
                # (bc = [I64|I64]; rows 0-63 rewritten with equal values)
                def trig_bcast(seg):
                    def go():
                        bp = ps_s.tile([128, 1024], f32, tag="s")
                        for j in range(2):
                            nc.tensor.matmul(
                                bp[:, j * 512 : j * 512 + 512],
                                bc[:],
                                trig[
                                    0:64,
                                    seg * 1024 + j * 512 : seg * 1024 + j * 512 + 512,
                                ],
                                start=True, stop=True,
                            )
                        nc.scalar.copy(
                            trig[:, seg * 1024 : seg * 1024 + 1024], bp[:]
                        )
                    return go

                def phase1_groups(ci, pair, load_x):
                    """Issue-groups for (chunk ci, head-pair pair)'s
                    QKV+RoPE+RMS. x chunk is shared: loaded once (pair 0)."""
                    t0 = ci * CH
                    state = {}

                    if load_x:
                        xt = xp.tile([128, 8, CH], f16, tag="x")
                        xsrc = xT_d[:, t0 : t0 + CH].rearrange(
                            "(g p) t -> p g t", p=128
                        )
                        nc.sync.dma_start(xt[:, 0:4, :], xsrc[:, 0:4, :])
                        nc.sync.dma_start(xt[:, 4:8, :], xsrc[:, 4:8, :])
                        state["xt"] = xt
                        _xt_share[ci] = xt
                    else:
                        state["xt"] = _xt_share[ci]

                    def tslc(w):
                        # shared trig table: 0=cos 1=sin (norm weight is
                        # applied to the rsqrt factor instead)
                        return trig[:, w * T + t0 : w * T + t0 + CH]

                    def proj_a(w_sb, name):
                        def go():
                            ps = ps_c.tile([128, CH], f32, tag="c")
                            state[name] = ps
                            for cc in range(4):
                                nc.tensor.matmul(
                                    ps[:],
                                    w_sb[:, pair * C + 128 * cc : pair * C + 128 * cc + 128],
                                    state["xt"][:, cc, :],
                                    start=(cc == 0), stop=False,
                                )
                        return go

                    def proj(w_sb, name):
                        def go():
                            ps = state[name]
                            for cc in range(4, 8):
                                nc.tensor.matmul(
                                    ps[:],
                                    w_sb[:, pair * C + 128 * cc : pair * C + 128 * cc + 128],
                                    state["xt"][:, cc, :],
                                    start=False, stop=(cc == 7),
                                )
                            if name != "v":
                                cw = 0
                                xs = scr.tile([128, CH], f32, tag="s")
                                nc.vector.stream_shuffle(xs[:], ps[:], SHUF_MASK)
                                tc_ = scr.tile([128, CH], f32, tag="s")
                                nc.vector.tensor_mul(tc_[:], ps[:], tslc(cw))
                                # sum of squares is pair-permutation
                                # invariant: square the shuffled SBUF copy
                                sq = scr.tile([128, CH], f32r, tag="s")
                                with nc.allow_low_precision(reason="rms f32r"):
                                    nc.vector.tensor_mul(sq[:], xs[:], xs[:])
                                ts2 = scr.tile([128, CH], f32, tag="s")
                                nc.gpsimd.tensor_mul(ts2[:], xs[:], tslc(cw + 1))
                                o_ = scr.tile([128, CH], f32, tag="s")
                                nc.gpsimd.tensor_add(o_[:], tc_[:], ts2[:])
                                state[name + "_sq"] = sq
                                state[name + "_o"] = o_
                            else:
                                v_sb = scrv.tile([128, CH], f16, tag="v")
                                if (2 * ci + pair + 3) % 4 < 2:
                                    nc.scalar.copy(v_sb[:], ps[:])
                                else:
                                    nc.vector.tensor_copy(v_sb[:], ps[:])
                                state["v_sb"] = v_sb

                        return go

                    def rms_and_vt():
                        # block-identity matmul broadcasts sum(q^2) per head;
                        # q and k share one tile: ln/exp run once, full width
                        ms_ps = ps_s.tile([128, 2 * CH], f32, tag="s")
                        for i, name in enumerate(("q", "k")):
                            nc.tensor.matmul(
                                ms_ps[:, i * CH : i * CH + CH],
                                ee[:], state[name + "_sq"][:],
                                start=True, stop=True,
                            )
                        state["ms"] = ms_ps
                        for ti in range(4):
                            vt_ps = ps_c.tile([128, 128], f16, tag="c")
                            nc.tensor.transpose(
                                vt_ps[:],
                                state["v_sb"][:, ti * 128 : ti * 128 + 128],
                                ident[:],
                            )
                            kb = pair * NKT + (t0 // KT) + ti
                            dst = vaug[:, kb * 130 : kb * 130 + 130].rearrange(
                                "p (k c) -> p k c", c=65
                            )[:, :, 0:64]
                            nc.vector.tensor_copy(
                                dst, vt_ps[:].rearrange("p (k c) -> p k c", c=64)
                            )

                    def norm_store():
                        ln_ = scr2.tile([128, 2 * CH], f32, tag="s2")
                        nc.scalar.activation(
                            ln_[:], state["ms"][:], AF.Ln,
                            scale=1.0 / D, bias=epsb[:],
                        )
                        rs_ = scr2.tile([128, 2 * CH], f32, tag="s2")
                        nc.scalar.activation(rs_[:], ln_[:], AF.Exp, scale=-0.5)
                        for i, (name, dst) in enumerate((("q", qT), ("k", kT))):
                            # fold the rmsnorm weight (per-partition scalar)
                            # into the rsqrt factor in place
                            nc.gpsimd.tensor_scalar_mul(
                                rs_[:, i * CH : i * CH + CH],
                                rs_[:, i * CH : i * CH + CH],
                                wnorm[:, i : i + 1],
                            )
                            nc.vector.tensor_mul(
                                dst[:, pair * T + t0 : pair * T + t0 + CH],
                                state[name + "_o"][:],
                                rs_[:, i * CH : i * CH + CH],
                            )

                    return [
                        proj_a(wq_sb, "q"),
                        proj(wq_sb, "q"),
                        proj_a(wk_sb, "k"),
                        proj(wk_sb, "k"),
                        proj_a(wv_sb, "v"),
                        proj(wv_sb, "v"),
                        rms_and_vt,
                        norm_store,
                    ]

                def attn(pair, qi, fillers):
                    """Attention k-loop for (pair, qi), popping one filler
                    closure after each (s_exp, pv) pair so PE always has
                    independent work queued behind the exp-gated PV
                    matmuls."""
                    bt = pair * T
                    q0 = bt + qi * QT
                    nk = 4 * qi + 4
                    y_ps = ps_y.tile([65, 2 * QT], f32, tag="y")
                    p_tiles = [None] * nk
                    fillers = list(fillers)

                    def fill():
                        if fillers:
                            fillers.pop(0)()

                    def s_exp(ki):
                        mi = ki - 4 * qi
                        off = max(mi, 0) * KT
                        diag = mi >= 0
                        k0 = bt + ki * KT
                        s_ps = ps_s.tile([128, 2 * QT], f32, tag="s")
                        for h in range(2):
                            hs = slice(64 * h, 64 * h + 64)
                            nc.tensor.matmul(
                                s_ps[:, h * QT + off : h * QT + QT],
                                kT[hs, k0 : k0 + KT],
                                qT[hs, q0 + off : q0 + QT],
                                start=True, stop=not diag,
                                tile_position=(64 * h, 0),
                            )
                        if diag:
                            # causal mask: accumulate NEG strict-lower
                            # triangle into the diagonal block; exp -> 0
                            for h in range(2):
                                nc.tensor.matmul(
                                    s_ps[:, h * QT + off : h * QT + off + KT],
                                    mtri[:], ident[:],
                                    start=False, stop=True,
                                )
                        p_sb = pbuf.tile([128, 2 * QT], f16, tag="p")
                        if off == 0:
                            nc.scalar.activation(
                                p_sb[:], s_ps[:], AF.Exp, scale=SC
                            )
                        else:
                            nc.scalar.activation(
                                p_sb[:].rearrange("p (h q) -> p h q", h=2)[
                                    :, :, off:QT
                                ],
                                s_ps[:].rearrange("p (h q) -> p h q", h=2)[
                                    :, :, off:QT
                                ],
                                AF.Exp, scale=SC,
                            )
                        p_tiles[ki] = p_sb

                    def pv(ki):
                        mi = ki - 4 * qi
                        off = max(mi, 0) * KT
                        kb = pair * NKT + ki
                        for h in range(2):
                            nc.tensor.matmul(
                                y_ps[:, h * QT + off : h * QT + QT],
                                vaug[
                                    :, kb * 130 + 65 * h : kb * 130 + 65 * h + 65
                                ],
                                p_tiles[ki][:, h * QT + off : h * QT + QT],
                                start=(ki == 0), stop=(ki == nk - 1),
                            )

                    # spread fillers evenly over the k-loop's fill slots
                    nslots = nk + 1
                    nf = len(fillers)
                    sched = {}
                    for i in range(nf):
                        s = min(nslots - 1, (i * nslots) // max(nf, 1))
                        sched[s] = sched.get(s, 0) + 1
                    slot = 0

                    def fill_at():
                        nonlocal slot
                        for _ in range(sched.get(slot, 0)):
                            fill()
                        slot += 1

                    s_exp(0)
                    if nk > 1:
                        s_exp(1)
                    fill_at()
                    for ki in range(2, nk):
                        s_exp(ki)
                        fill_at()
                        pv(ki - 2)
                    fill_at()
                    pv(nk - 2) if nk > 1 else None
                    pv(nk - 1)
                    while fillers:
                        fill()

                    # free y_ps immediately; normalization defers as fillers
                    yraw = ysc.tile([65, 2 * QT], f32r, tag="yraw")
                    nc.scalar.copy(yraw[64:65, :], y_ps[64:65, :])
                    if qi == 3 and pair == 1:
                        nc.scalar.copy(yraw[0:64, :], y_ps[0:64, :])
                    else:
                        nc.vector.tensor_copy(yraw[0:64, :], y_ps[0:64, :])
                    rcp = ysc.tile([1, 2 * QT], f32r, tag="rcp")
                    with nc.allow_low_precision(reason="denominator recip"):
                        nc.vector.reciprocal(rcp[:], yraw[64:65, :])

                    def tail_norm():
                        db_ps = ps_y.tile([64, 2 * QT], f32, tag="y")
                        for h in range(2):
                            nc.tensor.matmul(
                                db_ps[:, h * QT : h * QT + QT],
                                onesc[:],
                                rcp[:, h * QT : h * QT + QT],
                                start=True, stop=True,
                            )
                        yTq = ysc.tile([128, QT], f16, tag="yTq")
                        for h in range(2):
                            nc.vector.tensor_mul(
                                yTq[64 * h : 64 * h + 64, :],
                                yraw[0:64, h * QT : h * QT + QT],
                                db_ps[:, h * QT : h * QT + QT],
                            )
                        yTq_store[pair] = yTq

                    return tail_norm

                def tail_proj(qi, yTq0, yTq1, last):
                    """Out-projection for q-tile qi: both pairs chained into
                    one PSUM accumulation (contraction 256), single copy and
                    one DMA per 128-token block."""
                    def go():
                        q0 = qi * QT
                        for tt in range(4):
                            o_sb = osb.tile([128, C], f16, tag="o")
                            for half in range(2):
                                o_ps = ps_c.tile([128, 512], f32, tag="c")
                                nc.tensor.matmul(
                                    o_ps[:],
                                    yTq0[:, tt * 128 : tt * 128 + 128],
                                    wo_sb[:, half * 512 : half * 512 + 512],
                                    start=True, stop=False,
                                )
                                nc.tensor.matmul(
                                    o_ps[:],
                                    yTq1[:, tt * 128 : tt * 128 + 128],
                                    wo_sb[:, C + half * 512 : C + half * 512 + 512],
                                    start=False, stop=True,
                                )
                                dsl = o_sb[:, half * 512 : half * 512 + 512]
                                if half == 0 or last:
                                    nc.scalar.copy(dsl, o_ps[:])
                                else:
                                    nc.vector.tensor_copy(dsl, o_ps[:])
                            nc.gpsimd.dma_start(
                                out_d[q0 + tt * 128 : q0 + tt * 128 + 128, :],
                                o_sb[:],
                            )
                    return go

                _xt_share = {}
                yTq_store = {}

                # startup: chunk 0 for both pairs runs un-overlapped, with
                # the trig broadcast segments woven in front (tables must
                # exist before the rope muls)
                for g in [trig_bcast(s) for s in range(4)]:
                    g()
                for g in phase1_groups(0, 0, True):
                    g()
                for g in phase1_groups(0, 1, False):
                    g()

                # steady state: per chunk ci, attention for both pairs at
                # q-tile ci; phase1 for chunk ci+1 and the previous q-tile's
                # tails thread in as PE filler.
                tails = []      # deferred closures from previous iterations
                for ci in range(NCH):
                    nxt = (
                        phase1_groups(ci + 1, 0, True)
                        + phase1_groups(ci + 1, 1, False)
                        if ci + 1 < NCH
                        else []
                    )
                    # attn(pair0): fillers = tails + first half of phase1
                    f0 = tails + nxt[0:8]
                    tn0 = attn(0, ci, f0)
                    # attn(pair1): fillers = pair0's tail_norm + rest
                    f1 = [tn0] + nxt[8:16]
                    tn1 = attn(1, ci, f1)
                    # defer pair1 tail_norm + fused out-projection into the
                    # next iteration's pair-0 k-loop (it is long enough)
                    y0ref = []

                    def mk_tail(ci=ci, tn1=tn1):
                        done = {}

                        def run_tn1():
                            tn1()
                            done["y0"] = yTq_store[0]
                            done["y1"] = yTq_store[1]

                        def run_proj():
                            tail_proj(
                                ci, done["y0"], done["y1"], ci == NCH - 1
                            )()

                        return [run_tn1, run_proj]

                    tails = mk_tail()
                for g in tails:
                    g()

    nc.finalize()
    return nc


def _host_prep(x, w_qkv, w_out, q_norm_w, k_norm_w):
    j = np.arange(32, dtype=np.float64)
    inv = ROPE_BASE ** (-j / 32.0)
    tt = np.arange(T, dtype=np.float64)
    ang = tt[:, None] * inv[None, :]
    cos_t = np.cos(ang)
    sin_t = np.sin(ang)

    # d-channel order per head: rope pair (d, d+32) sits 16 partitions
    # apart within one 32-partition quadrant (stream_shuffle i^16 reaches it)
    dmap = np.r_[0:16, 32:48, 16:32, 48:64]

    # shared (q/k) tables: rope sign structure only, no norm-weight fold
    cosr = np.empty((64, T), np.float64)
    sinr = np.empty((64, T), np.float64)
    for p in range(64):
        d = dmap[p]
        jj = d % 32
        sign = -1.0 if d < 32 else 1.0
        cosr[p] = cos_t[:, jj]
        sinr[p] = sign * sin_t[:, jj]
    trig = np.concatenate([cosr, sinr], axis=1).astype(F16)

    # per-partition norm weights (dmap order, repeated for both 64-halves)
    wn = np.empty((128, 2), np.float32)
    wn[:, 0] = np.asarray(q_norm_w, np.float64)[dmap[np.arange(128) % 64]]
    wn[:, 1] = np.asarray(k_norm_w, np.float64)[dmap[np.arange(128) % 64]]

    bc = np.zeros((64, 128), np.float32)
    bc[np.arange(64), np.arange(64)] = 1.0
    bc[np.arange(64), 64 + np.arange(64)] = 1.0

    mtri = np.where(
        np.arange(128)[:, None] < np.arange(128)[None, :], NEG, 0.0
    ).astype(F16)
    ee = np.zeros((128, 128), np.float32)
    ee[0:64, 0:64] = 1.0
    ee[64:128, 64:128] = 1.0
    ident = np.eye(128, dtype=np.float32).astype(F16)

    shared = {
        "trig": trig, "wnorm": wn, "bc": bc.astype(F16), "mtri": mtri,
        "ee": ee,
        "ident": ident,
        "onesc": np.ones((1, 64), np.float32),
        "epsb": np.full((128, 1), EPS, np.float32),
    }

    xTb = [
        np.ascontiguousarray(x[b].T.astype(F16)) for b in range(B)
    ]

    in_maps = []
    for c in range(NC):
        bb = c // 4
        hg = c % 4
        m = dict(shared)
        m["xT"] = xTb[bb]
        # two head-pairs: rows for pair p = heads (4hg+2p, 4hg+2p+1)
        qk_rows = []
        v_rows = []
        for p in range(2):
            h0 = (4 * hg + 2 * p) * 64
            rows = np.arange(h0, h0 + 128)
            qk_rows.append(np.concatenate([rows[0:64][dmap], rows[64:128][dmap]]))
            v_rows.append(rows)
        qk_rows = np.concatenate(qk_rows)   # [256]
        v_rows = np.concatenate(v_rows)     # [256]
        # proj matmul lhsT layout: wq_sb[p_in, q*C + g*128 + i_out]
        #   = w_qkv[row_set[q*128 + i_out], g*128 + p_in]
        def sbuf_w(wmat, rows):
            arr = wmat[rows, :].reshape(2, 128, 8, 128)  # [q, i, g, p]
            return np.ascontiguousarray(
                arr.transpose(3, 0, 2, 1).reshape(128, 2 * C).astype(F16)
            )

        m["wqT"] = sbuf_w(w_qkv, qk_rows)
        m["wkT"] = sbuf_w(w_qkv, C + qk_rows)
        m["wvT"] = sbuf_w(w_qkv, 2 * C + v_rows)
        # wo_sb[p, q*C + c] = w_out[c, v_rows[q*128 + p]]
        m["woT"] = np.ascontiguousarray(
            w_out[:, v_rows].T.reshape(2, 128, C).transpose(1, 0, 2)
            .reshape(128, 2 * C).astype(F16)
        )
        in_maps.append(m)
    return in_maps


def kernel(x, w_qkv, w_out, q_norm_w, k_norm_w, _trace=False):
    from concourse.bass_utils import run_bass_kernel_spmd

    if "nc" not in _cache:
        _cache["nc"] = _build()
    nc = _cache["nc"]

    x = np.asarray(x, dtype=np.float32)
    w_qkv = np.asarray(w_qkv, dtype=np.float32)
    w_out = np.asarray(w_out, dtype=np.float32)
    q_norm_w = np.asarray(q_norm_w, dtype=np.float32)
    k_norm_w = np.asarray(k_norm_w, dtype=np.float32)

    in_maps = _host_prep(x, w_qkv, w_out, q_norm_w, k_norm_w)
    res = run_bass_kernel_spmd(nc, in_maps, list(range(NC)), trace=_trace)
    _cache["last_result"] = res
    out = np.zeros((B, T, C), np.float64)
    for c, r in enumerate(res.results):
        out[c // 4] += np.asarray(r["out"]).astype(np.float64)
    return out.astype(np.float32)
